# revision 9
# baseline (speedup 1.0000x reference)
"""Trainium2 Bass kernel for nn_Matcher (retrieval_knn attention).

Math (per object o, with S=1 batch):
  logits[b,n] = (keys[o,:,b] . q_in[:,n]) / sqrt(Dk)
  p           = softmax_b(logits)
  mem[v,n]    = sum_b values[o,v,b] p[b,n]
  maskmem[n]  = sum_b masks[o,b] p[b,n]
  out[o]      = concat([mem, q_out * maskmem], axis=0)   # [1024, n]

Sharding: 8 cores = 4 objects x 2 query halves (n in [0,1800) / [1800,3600)).
No cross-core communication.

Per-core kernel strategy (v2, fp8 DoubleRow):
  mm1: S[b,n] = keys_chunk^T @ q_in  (fp32r, exact logits), pairs of bank
       chunks into one 2-bank PSUM tile.
  exp: E8 = fp8_e4m3(exp(S/sqrt(Dk) - SHIFT))  (ACT, one instr per pair,
       SHIFT keeps exp in fp8 range; softmax is shift-invariant).
  mm2: acc[v',n] += vhat8_pair^T @ E8_pair  -- fp8 DoubleRow perf mode:
       each matmul consumes TWO 128-bank chunks at 0.5 cyc/row.
       vhat8 = [values^T | mask | ones] : [7424, 514] fp8.
       4 value streams (4 PSUM banks) + a separate [2,nw] mask/denom
       accumulation sweep whose PSUM lives in the s-pool rotation.
  Normalize: md2=[maskraw;denom] (bf16), broadcast via K<=2 matmuls,
       reciprocal + muls on DVE, outputs in bf16 (host converts to fp32).
"""

import sys

sys.path.insert(0, "/opt/trn_rl_repo")

import numpy as np
import ml_dtypes

OBJ_N, D_KEY, D_VAL, BANK_N, N_Q = 4, 128, 512, 7200, 3600
N_CORES = 8
N_HALF = N_Q // 2            # 1800 queries per core
P = 128
NB = (BANK_N + P - 1) // P   # 57 bank chunks (56 x 128 + 1 x 32)
NB2 = NB + 1                 # 58: padded to even for DoubleRow pairs
NPAIR = NB2 // 2             # 29
B_PAD = NB2 * P              # 7424
LAST_BW = BANK_N - (NB - 1) * P  # 32
VHAT_W = D_VAL + 2           # 514: values^T | mask | ones
SCALE = 1.0 / float(np.sqrt(D_KEY))
SHIFT = 2.5                  # exp(z-SHIFT): keeps e in fp8_e4m3 range
NW = 450                     # uniform query-strip width (4 x 450 = 1800)
SW_ILV = False               # DoubleRowSwInterleave (contiguous weight reads)
N_CHUNKS = [(i * NW, NW) for i in range(4)]
DMA_GROUP = 8                # bank chunks per bulk DMA

_CACHE = {}


def _build(reps=1, bench=False, reload_in_rep=True):
    import concourse.bacc as bacc
    import concourse.mybir as mybir
    import concourse.tile as tile

    f32 = mybir.dt.float32
    f32r = mybir.dt.float32r
    bf16 = mybir.dt.bfloat16
    f8 = mybir.dt.float8e4
    u8 = mybir.dt.uint8
    Exp = mybir.ActivationFunctionType.Exp
    DR = (mybir.MatmulPerfMode.DoubleRowSwInterleave if SW_ILV
          else mybir.MatmulPerfMode.DoubleRow)

    nc = bacc.Bacc("TRN2", target_bir_lowering=False, debug=False)

    ikind = {} if bench else {"kind": "ExternalInput"}
    okind = {} if bench else {"kind": "ExternalOutput"}
    consts_d = nc.dram_tensor("consts", [2, P], bf16, kind="ExternalInput")
    keys_d = nc.dram_tensor("keys", [D_KEY, NB * P], f32r, **ikind)
    vals_d = nc.dram_tensor("vals", [P, NPAIR * 4 * 2 * P], u8, **ikind)
    mo_d = nc.dram_tensor("mo", [P, NPAIR * 2 * 16], u8, **ikind)
    zeros_d = nc.dram_tensor("zeros", [P, 512], u8, kind="ExternalInput")
    qin_d = nc.dram_tensor("qin", [D_KEY, N_HALF], f32r, **ikind)
    qout_d = nc.dram_tensor("qout", [D_VAL, N_HALF], bf16, **ikind)
    out_d = nc.dram_tensor("out", [2 * D_VAL, N_HALF], bf16, **okind)
    if bench:
        dout_d = nc.dram_tensor("dout", [1, P], bf16, kind="ExternalOutput")

    keys_ap = keys_d.ap().rearrange("d (c q) -> d c q", q=P)        # [128, 57, 128]
    vals_ap = vals_d.ap().rearrange("p (c m q) -> p c m q",
                                    c=NPAIR, m=4)        # [128, 29, 4, 256]
    mo_ap = mo_d.ap().rearrange("p (c j) -> p c j", c=NPAIR)   # [128, 29, 32]
    qout_ap = qout_d.ap().rearrange("(r p) n -> p r n", p=P)        # [128, 4, 1800]
    out_ap = out_d.ap().rearrange("(r p) n -> p r n", p=P)          # [128, 8, 1800]

    with tile.TileContext(nc) as tc:
        with (
            tc.tile_pool(name="persist", bufs=1) as persist,
            tc.tile_pool(name="qin_p", bufs=2) as qin_p,
            tc.tile_pool(name="qout_p", bufs=2) as qout_p,
            tc.tile_pool(name="row_p", bufs=2) as row_p,
            tc.tile_pool(name="bcsb_p", bufs=2) as bcsb_p,
            tc.tile_pool(name="out_p", bufs=6) as out_p,
            tc.tile_pool(name="s_ps", bufs=2, space="PSUM") as s_ps,
            tc.tile_pool(name="acc_ps", bufs=1, space="PSUM") as acc_ps,
        ):
            # Persistent operands
            keys_sb = persist.tile([P, NB, P], f32r)
            vals_sb = persist.tile([P, NPAIR, 4, 2 * P], u8)
            vals_f8 = vals_sb.bitcast(f8)
            mo_sb = persist.tile([P, NPAIR, 32], u8)
            mo_f8 = mo_sb.bitcast(f8)
            e8_sb = persist.tile([P, NB2, NW], f8)
            ones_col = persist.tile([1, P], bf16)
            nc.vector.memset(ones_col[:], 1.0)
            shift_sb = persist.tile([P, 1], f32)
            nc.vector.memset(shift_sb[:], -SHIFT)
            # Warm the ACT exp table so the first real exp doesn't pay the
            # ACT_TABLE_LOAD on the critical path.
            warm = persist.tile([1, 1], f32)
            nc.vector.memset(warm[:], 0.0)
            nc.scalar.activation(warm[:], warm[:], Exp, scale=1.0)
            sel2 = persist.tile([2, P], bf16)  # row0=0, row1=1 (selects denom)
            nc.sync.dma_start(sel2[:], consts_d.ap()[:, :])
            # One-time zeroing of the e8 pad regions (DMA from zeroed DRAM;
            # engines may not address partition offsets).
            nc.sync.dma_start(e8_sb[:, NB2 - 1:NB2, :].bitcast(u8),
                              zeros_d.ap()[:, 0:NW])
            nc.sync.dma_start(e8_sb[LAST_BW:, NB - 1:NB, :].bitcast(u8),
                              zeros_d.ap()[LAST_BW:, 0:NW])

            def bulk_load(first_rep):
                n0_0, nw_0 = N_CHUNKS[0]
                qin_t0 = qin_p.tile([P, nw_0], f32r, tag="qin", name="qin_t0")
                nc.sync.dma_start(qin_t0[:], qin_d.ap()[:, n0_0:n0_0 + nw_0])
                qout_t0 = None
                g0 = 0
                for gsz in [1, 1, 2, 4] + [DMA_GROUP] * NB:
                    if g0 >= NB:
                        break
                    g1 = min(g0 + gsz, NB)
                    if first_rep:
                        nc.sync.dma_start(keys_sb[:, g0:g1, :], keys_ap[:, g0:g1, :])
                        p0, p1 = g0 // 2, min((g1 + 1) // 2, NPAIR)
                        if p1 > p0:
                            nc.sync.dma_start(vals_sb[:, p0:p1, :, :],
                                              vals_ap[:, p0:p1, :, :])
                            nc.sync.dma_start(mo_sb[:, p0:p1, :],
                                              mo_ap[:, p0:p1, :])
                    g0 = g1
                    if g0 == 8:
                        qout_t0 = qout_p.tile([P, D_VAL // P, nw_0], bf16,
                                              tag="qout", name="qout_t0")
                        nc.sync.dma_start(qout_t0[:], qout_ap[:, :, n0_0:n0_0 + nw_0])
                return qin_t0, qout_t0

            for _rep in range(reps):
                qin_t0, qout_t0 = bulk_load(reload_in_rep or _rep == 0)

                for j, (n0, nw) in enumerate(N_CHUNKS):
                    if j == 0:
                        qin_t, qout_t = qin_t0, qout_t0
                    else:
                        qin_t = qin_p.tile([P, nw], f32r, tag="qin")
                        nc.sync.dma_start(qin_t[:], qin_d.ap()[:, n0:n0 + nw])
                        qout_t = qout_p.tile([P, D_VAL // P, nw], bf16, tag="qout")
                        nc.sync.dma_start(qout_t[:], qout_ap[:, :, n0:n0 + nw])

                    accs = [
                        acc_ps.tile([P, nw], f32, tag=f"acc{m}", name=f"acc{m}")
                        for m in range(4)
                    ]

                    # Software pipeline: mm1 pair + exp for pair pc, then the
                    # DoubleRow mm2 burst for pair pc-1 (so exp hides under
                    # PE's mm2 work).
                    for pc in range(NPAIR + 1):
                        if pc < NPAIR:
                            c0 = 2 * pc
                            s_t = s_ps.tile([P, 2, NW], f32, tag="s",
                                            padded_shape=[P, 2, 512])
                            nc.tensor.matmul(
                                s_t[:, 0, :nw], keys_sb[:, c0, :], qin_t[:],
                                start=True, stop=True,
                            )
                            if c0 + 1 < NB:
                                bw1 = P if c0 + 1 < NB - 1 else LAST_BW
                                nc.tensor.matmul(
                                    s_t[:bw1, 1, :nw], keys_sb[:, c0 + 1, :bw1],
                                    qin_t[:], start=True, stop=True,
                                )
                                nc.scalar.activation(
                                    e8_sb[:, c0:c0 + 2, :nw], s_t[:, :, :nw],
                                    Exp, scale=SCALE, bias=shift_sb[:, :])
                            else:
                                # last pair: chunk 56 is 32 banks; slot 57 and
                                # banks 32: of slot 56 are zero (one-time DMA)
                                nc.scalar.activation(
                                    e8_sb[:LAST_BW, c0:c0 + 1, :nw],
                                    s_t[:LAST_BW, 0:1, :nw],
                                    Exp, scale=SCALE, bias=shift_sb[:LAST_BW, :])
                        if pc > 0:
                            pq = pc - 1
                            pp = 2 * pq
                            for m in range(4):
                                nc.tensor.matmul(
                                    accs[m][:, :nw],
                                    (vals_f8[:, pq, m, :] if SW_ILV else
                                     vals_f8[:, pq, m, :].rearrange(
                                         "k (i q) -> k i q", i=2)),
                                    e8_sb[:, pp:pp + 2, :nw],
                                    start=(pc == 1), stop=(pc == NPAIR),
                                    perf_mode=DR,
                                )

                    # mask/denom accumulation sweep (PSUM slot from s-pool)
                    acc5_t = s_ps.tile([P, 2, NW], f32, tag="s", name="acc5",
                                       padded_shape=[P, 2, 512])
                    for pq in range(NPAIR):
                        pp = 2 * pq
                        nc.tensor.matmul(
                            acc5_t[0:16, 0, :nw] if SW_ILV else acc5_t[0:2, 0, :nw],
                            (mo_f8[:, pq, 0:32] if SW_ILV else
                             mo_f8[:, pq, :].rearrange(
                                 "k (i j) -> k i j", i=2)[:, :, 0:2]),
                            e8_sb[:, pp:pp + 2, :nw],
                            start=(pq == 0), stop=(pq == NPAIR - 1),
                            perf_mode=DR,
                        )

                    # Normalization: md2 rows = [maskraw, denom] (bf16)
                    md2 = row_p.tile([2, nw], bf16, tag="md2")
                    nc.scalar.copy(md2[:], acc5_t[0:2, 0, :nw])

                    bc_t = s_ps.tile([P, 2, NW], f32, tag="s", name="bc",
                                     padded_shape=[P, 2, 512])
                    nc.tensor.matmul(bc_t[:, 0, :nw], sel2[:], md2[:],
                                     start=True, stop=True)
                    rb_sb = bcsb_p.tile([P, nw], f32, tag="rb")
                    nc.vector.reciprocal(rb_sb[:], bc_t[:, 0, :nw])

                    nc.tensor.matmul(bc_t[:, 1, :nw], ones_col[:], md2[0:1, :],
                                     start=True, stop=True)
                    mn_sb = bcsb_p.tile([P, nw], f32, tag="mn")
                    nc.vector.tensor_mul(mn_sb[:], bc_t[:, 1, :nw], rb_sb[:])

                    for m in range(4):
                        o_t = out_p.tile([P, nw], bf16, tag="out")
                        nc.vector.tensor_mul(o_t[:], accs[m][:, :nw], rb_sb[:])
                        nc.sync.dma_start(out_ap[:, m, n0:n0 + nw], o_t[:])
                    for m in range(4):
                        o_t = out_p.tile([P, nw], bf16, tag="out")
                        nc.vector.tensor_mul(o_t[:], qout_t[:, m, :], mn_sb[:])
                        nc.sync.dma_start(out_ap[:, 4 + m, n0:n0 + nw], o_t[:])

            if bench:
                dsb = persist.tile([1, P], bf16)
                nc.vector.tensor_copy(dsb[:], ones_col[:])
                nc.sync.dma_start(dout_d.ap()[:, :], dsb[:])

    nc.compile()
    return nc


def _get_nc():
    if "nc" not in _CACHE:
        _CACHE["nc"] = _build()
    return _CACHE["nc"]


def _get_runner():
    """Build the multi-core PJRT runner once (mirrors bass2jax.run_bass_via_pjrt)."""
    if "runner" in _CACHE:
        return _CACHE["runner"]
    import jax
    from jax.sharding import Mesh, PartitionSpec
    from jax.experimental.shard_map import shard_map
    import concourse.mybir as mybir
    from concourse import bass2jax
    from concourse.bass2jax import _bass_exec_p, install_neuronx_cc_hook

    nc = _get_nc()
    install_neuronx_cc_hook()
    partition_name = nc.partition_id_tensor.name if nc.partition_id_tensor else None
    in_names, out_names, out_avals = [], [], []
    for alloc in nc.m.functions[0].allocations:
        if not isinstance(alloc, mybir.MemoryLocationSet):
            continue
        name = alloc.memorylocations[0].name
        if alloc.kind == "ExternalInput":
            if name != partition_name:
                in_names.append(name)
        elif alloc.kind == "ExternalOutput":
            out_names.append(name)
            out_avals.append(jax.core.ShapedArray(
                tuple(alloc.tensor_shape), mybir.dt.np(alloc.dtype)))
    n_params = len(in_names)
    zero_outs = [np.zeros(a.shape, a.dtype) for a in out_avals]
    all_in_names = list(in_names) + list(out_names)
    if partition_name is not None:
        all_in_names.append(partition_name)

    def _body(*args):
        operands = list(args)
        if partition_name is not None:
            operands.append(bass2jax.partition_id_tensor())
        outs = _bass_exec_p.bind(
            *operands,
            out_avals=tuple(out_avals),
            in_names=tuple(all_in_names),
            out_names=tuple(out_names),
            lowering_input_output_aliases=(),
            sim_require_finite=True,
            sim_require_nnan=True,
            nc=nc,
        )
        return tuple(outs)

    try:
        devices = jax.devices("axon")
    except Exception:
        devices = [d for d in jax.devices() if d.platform != "cpu"] or jax.devices()
    devices = devices[:N_CORES]
    assert len(devices) >= N_CORES, f"need {N_CORES} cores, got {len(devices)}"
    mesh = Mesh(np.asarray(devices), ("core",))
    n_io = n_params + len(out_names)
    fn = jax.jit(
        shard_map(_body, mesh=mesh,
                  in_specs=(PartitionSpec("core"),) * n_io,
                  out_specs=(PartitionSpec("core"),) * len(out_names),
                  check_rep=False),
        keep_unused=True)

    def run(in_maps):
        concat_in = [
            np.concatenate([np.asarray(m[name]) for m in in_maps], axis=0)
            for name in in_names
        ]
        concat_zero = [
            np.zeros((N_CORES * z.shape[0], *z.shape[1:]), z.dtype)
            for z in zero_outs
        ]
        out_arrs = fn(*concat_in, *concat_zero)
        return [
            {name: np.asarray(out_arrs[i]).reshape(N_CORES, *out_avals[i].shape)[c]
             for i, name in enumerate(out_names)}
            for c in range(N_CORES)
        ]

    _CACHE["runner"] = run
    return run


def kernel(keys, values, masks, q_in, q_out):

    keys = np.ascontiguousarray(np.asarray(keys, dtype=np.float32))
    values = np.asarray(values, dtype=np.float32)
    masks = np.asarray(masks, dtype=np.float32)
    q_in = np.ascontiguousarray(np.asarray(q_in, dtype=np.float32))
    q_out = np.asarray(q_out, dtype=np.float32)

    f8 = ml_dtypes.float8_e4m3
    bf = ml_dtypes.bfloat16

    # Host-side layout prep (per object, shared by 2 cores)
    keys_pad = np.zeros((OBJ_N, D_KEY, NB * P), dtype=np.float32)
    keys_pad[:, :, :BANK_N] = keys
    vpad = np.zeros((OBJ_N, D_VAL, B_PAD), dtype=f8)
    vpad[:, :, :BANK_N] = values.astype(f8)
    a = vpad.reshape(OBJ_N, 4, P, NPAIR, 2, P)   # [o, m, q, pc, i, p]
    if SW_ILV:
        # S[p, pc, m, 2j+i] = W_i[p, 127-j]: interleaved, columns reversed
        vals8 = a[:, :, ::-1, :, :, :].transpose(0, 5, 3, 1, 2, 4)
    else:
        # vals8[o, p, pc, m, i, q] = values[o, m*128+q, (2*pc+i)*128 + p]
        vals8 = a.transpose(0, 5, 3, 1, 4, 2)
    vals8 = np.ascontiguousarray(vals8).reshape(OBJ_N, P, NPAIR * 4 * 2 * P)
    vals8 = vals8.view(np.uint8)
    mpad = np.zeros((OBJ_N, 2, B_PAD), dtype=f8)
    mpad[:, 0, :BANK_N] = masks[:, 0].astype(f8)
    mpad[:, 1, :BANK_N] = 1.0
    mr = mpad.reshape(OBJ_N, 2, NPAIR, 2, P)     # [o, row(0=mask,1=ones), pc, i, p]
    mo8 = np.zeros((OBJ_N, P, NPAIR, 32), dtype=f8)
    if SW_ILV:
        # S[0]=ones_t0 S[1]=ones_t1 S[2]=mask_t0 S[3]=mask_t1 (cols reversed)
        mo8[:, :, :, 0] = mr[:, 1, :, 0, :].transpose(0, 2, 1)
        mo8[:, :, :, 1] = mr[:, 1, :, 1, :].transpose(0, 2, 1)
        mo8[:, :, :, 2] = mr[:, 0, :, 0, :].transpose(0, 2, 1)
        mo8[:, :, :, 3] = mr[:, 0, :, 1, :].transpose(0, 2, 1)
    else:
        # [pc, i, j] blocks at stride 16: j=0 mask, j=1 ones
        mo8.reshape(OBJ_N, P, NPAIR, 2, 16)[:, :, :, :, 0:2] = (
            mr.transpose(0, 4, 2, 3, 1))
    mo8 = np.ascontiguousarray(mo8).reshape(OBJ_N, P, NPAIR * 32)
    mo8 = mo8.view(np.uint8)
    zeros8 = np.zeros((P, 512), dtype=np.uint8)

    consts = np.zeros((2, P), dtype=bf)
    consts[1, :] = 1.0
    q_out_bf = q_out.astype(bf)

    in_maps = []
    for core in range(N_CORES):
        o, half = divmod(core, 2)
        nsl = slice(half * N_HALF, (half + 1) * N_HALF)
        in_maps.append({
            "consts": consts,
            "keys": keys_pad[o],
            "vals": vals8[o],
            "mo": mo8[o],
            "zeros": zeros8,
            "qin": np.ascontiguousarray(q_in[0, :, nsl]),
            "qout": np.ascontiguousarray(q_out_bf[0, :, nsl]),
        })

    run = _get_runner()
    results = run(in_maps)

    out = np.empty((1, OBJ_N, 2 * D_VAL, N_Q), dtype=np.float32)
    for core in range(N_CORES):
        o, half = divmod(core, 2)
        nsl = slice(half * N_HALF, (half + 1) * N_HALF)
        out[0, o, :, nsl] = results[core]["out"].astype(np.float32)
    return out


# revision 10
# speedup vs baseline: 1.3873x; 1.3873x over previous
"""Trainium2 Bass kernel for nn_Matcher (retrieval_knn attention).

Math (per object o, with S=1 batch):
  logits[b,n] = (keys[o,:,b] . q_in[:,n]) / sqrt(Dk)
  p           = softmax_b(logits)
  mem[v,n]    = sum_b values[o,v,b] p[b,n]
  maskmem[n]  = sum_b masks[o,b] p[b,n]
  out[o]      = concat([mem, q_out * maskmem], axis=0)   # [1024, n]

Sharding: 8 cores = 4 objects x 2 query halves (n in [0,1800) / [1800,3600)).
No cross-core communication.

Per-core kernel strategy (v2, fp8 DoubleRow):
  mm1: S[b,n] = keys_chunk^T @ q_in  (fp32r, exact logits), pairs of bank
       chunks into one 2-bank PSUM tile.
  exp: E8 = fp8_e4m3(exp(S/sqrt(Dk) - SHIFT))  (ACT, one instr per pair,
       SHIFT keeps exp in fp8 range; softmax is shift-invariant).
  mm2: acc[v',n] += vals8_pair^T @ E8_pair  -- fp8 DoubleRow perf mode:
       each matmul consumes TWO 128-bank chunks (~1.4x bf16; LDWEIGHTS for
       the 256-col dual-fp8 load runs serially, which caps the gain).
       vals8 layout [p, pair, m, ktile, col] gives the pair-contiguous
       (stride%16==0) weights DoubleRow's ISA check requires; mask|ones
       live in a tiny separate mo8 array. 4 value streams (4 PSUM banks)
       + a [2,nw] mask/denom accumulation sweep whose PSUM slot comes from
       the s-pool rotation (PSUM budget: 4 acc + 2x2 s banks = 8).
  Normalize: md2=[maskraw;denom] (bf16), broadcast via K<=2 matmuls,
       reciprocal + muls on DVE, outputs in bf16 (host converts to fp32).
"""

import sys

sys.path.insert(0, "/opt/trn_rl_repo")

import numpy as np
import ml_dtypes

OBJ_N, D_KEY, D_VAL, BANK_N, N_Q = 4, 128, 512, 7200, 3600
N_CORES = 8
N_HALF = N_Q // 2            # 1800 queries per core
P = 128
NB = (BANK_N + P - 1) // P   # 57 bank chunks (56 x 128 + 1 x 32)
NB2 = NB + 1                 # 58: padded to even for DoubleRow pairs
NPAIR = NB2 // 2             # 29
B_PAD = NB2 * P              # 7424
LAST_BW = BANK_N - (NB - 1) * P  # 32
VHAT_W = D_VAL + 2           # 514: values^T | mask | ones
SCALE = 1.0 / float(np.sqrt(D_KEY))
SHIFT = 2.5                  # exp(z-SHIFT): keeps e in fp8_e4m3 range
NW = 450                     # uniform query-strip width (4 x 450 = 1800)
SW_ILV = False               # DoubleRowSwInterleave (contiguous weight reads)
N_CHUNKS = [(i * NW, NW) for i in range(4)]
DMA_GROUP = 8                # bank chunks per bulk DMA

_CACHE = {}


def _build(reps=1, bench=False, reload_in_rep=True):
    import concourse.bacc as bacc
    import concourse.mybir as mybir
    import concourse.tile as tile

    f32 = mybir.dt.float32
    f32r = mybir.dt.float32r
    bf16 = mybir.dt.bfloat16
    f8 = mybir.dt.float8e4
    u8 = mybir.dt.uint8
    Exp = mybir.ActivationFunctionType.Exp
    DR = (mybir.MatmulPerfMode.DoubleRowSwInterleave if SW_ILV
          else mybir.MatmulPerfMode.DoubleRow)

    nc = bacc.Bacc("TRN2", target_bir_lowering=False, debug=False)

    ikind = {} if bench else {"kind": "ExternalInput"}
    okind = {} if bench else {"kind": "ExternalOutput"}
    consts_d = nc.dram_tensor("consts", [2, P], bf16, kind="ExternalInput")
    keys_d = nc.dram_tensor("keys", [D_KEY, NB * P], f32r, **ikind)
    vals_d = nc.dram_tensor("vals", [P, NPAIR * 4 * 2 * P], u8, **ikind)
    mo_d = nc.dram_tensor("mo", [P, NPAIR * 2 * 16], u8, **ikind)
    zeros_d = nc.dram_tensor("zeros", [P, 512], u8, kind="ExternalInput")
    qin_d = nc.dram_tensor("qin", [D_KEY, N_HALF], f32r, **ikind)
    qout_d = nc.dram_tensor("qout", [D_VAL, N_HALF], bf16, **ikind)
    out_d = nc.dram_tensor("out", [2 * D_VAL, N_HALF], bf16, **okind)
    if bench:
        dout_d = nc.dram_tensor("dout", [1, P], bf16, kind="ExternalOutput")

    keys_ap = keys_d.ap().rearrange("d (c q) -> d c q", q=P)        # [128, 57, 128]
    vals_ap = vals_d.ap().rearrange("p (c m q) -> p c m q",
                                    c=NPAIR, m=4)        # [128, 29, 4, 256]
    mo_ap = mo_d.ap().rearrange("p (c j) -> p c j", c=NPAIR)   # [128, 29, 32]
    qout_ap = qout_d.ap().rearrange("(r p) n -> p r n", p=P)        # [128, 4, 1800]
    out_ap = out_d.ap().rearrange("(r p) n -> p r n", p=P)          # [128, 8, 1800]

    with tile.TileContext(nc) as tc:
        with (
            tc.tile_pool(name="persist", bufs=1) as persist,
            tc.tile_pool(name="qin_p", bufs=2) as qin_p,
            tc.tile_pool(name="qout_p", bufs=2) as qout_p,
            tc.tile_pool(name="row_p", bufs=2) as row_p,
            tc.tile_pool(name="bcsb_p", bufs=2) as bcsb_p,
            tc.tile_pool(name="out_p", bufs=6) as out_p,
            tc.tile_pool(name="s_ps", bufs=2, space="PSUM") as s_ps,
            tc.tile_pool(name="acc_ps", bufs=1, space="PSUM") as acc_ps,
        ):
            # Persistent operands
            keys_sb = persist.tile([P, NB, P], f32r)
            vals_sb = persist.tile([P, NPAIR, 4, 2 * P], u8)
            vals_f8 = vals_sb.bitcast(f8)
            mo_sb = persist.tile([P, NPAIR, 32], u8)
            mo_f8 = mo_sb.bitcast(f8)
            e8_sb = persist.tile([P, NB2, NW], f8)
            ones_col = persist.tile([1, P], bf16)
            nc.vector.memset(ones_col[:], 1.0)
            shift_sb = persist.tile([P, 1], f32)
            nc.vector.memset(shift_sb[:], -SHIFT)
            # Warm the ACT exp table so the first real exp doesn't pay the
            # ACT_TABLE_LOAD on the critical path.
            warm = persist.tile([1, 1], f32)
            nc.vector.memset(warm[:], 0.0)
            nc.scalar.activation(warm[:], warm[:], Exp, scale=1.0)
            sel2 = persist.tile([2, P], bf16)  # row0=0, row1=1 (selects denom)
            nc.sync.dma_start(sel2[:], consts_d.ap()[:, :])
            # One-time zeroing of the e8 pad regions (DMA from zeroed DRAM;
            # engines may not address partition offsets).
            nc.sync.dma_start(e8_sb[:, NB2 - 1:NB2, :].bitcast(u8),
                              zeros_d.ap()[:, 0:NW])
            nc.sync.dma_start(e8_sb[LAST_BW:, NB - 1:NB, :].bitcast(u8),
                              zeros_d.ap()[LAST_BW:, 0:NW])

            def bulk_load(first_rep):
                n0_0, nw_0 = N_CHUNKS[0]
                qin_t0 = qin_p.tile([P, nw_0], f32r, tag="qin", name="qin_t0")
                nc.sync.dma_start(qin_t0[:], qin_d.ap()[:, n0_0:n0_0 + nw_0])
                qout_t0 = None
                g0 = 0
                for gsz in [1, 1, 2, 4] + [DMA_GROUP] * NB:
                    if g0 >= NB:
                        break
                    g1 = min(g0 + gsz, NB)
                    if first_rep:
                        nc.sync.dma_start(keys_sb[:, g0:g1, :], keys_ap[:, g0:g1, :])
                        p0, p1 = g0 // 2, min((g1 + 1) // 2, NPAIR)
                        if p1 > p0:
                            nc.sync.dma_start(vals_sb[:, p0:p1, :, :],
                                              vals_ap[:, p0:p1, :, :])
                            nc.sync.dma_start(mo_sb[:, p0:p1, :],
                                              mo_ap[:, p0:p1, :])
                    g0 = g1
                    if g0 == 8:
                        qout_t0 = qout_p.tile([P, D_VAL // P, nw_0], bf16,
                                              tag="qout", name="qout_t0")
                        nc.sync.dma_start(qout_t0[:], qout_ap[:, :, n0_0:n0_0 + nw_0])
                return qin_t0, qout_t0

            for _rep in range(reps):
                qin_t0, qout_t0 = bulk_load(reload_in_rep or _rep == 0)

                for j, (n0, nw) in enumerate(N_CHUNKS):
                    if j == 0:
                        qin_t, qout_t = qin_t0, qout_t0
                    else:
                        qin_t = qin_p.tile([P, nw], f32r, tag="qin")
                        nc.sync.dma_start(qin_t[:], qin_d.ap()[:, n0:n0 + nw])
                        qout_t = qout_p.tile([P, D_VAL // P, nw], bf16, tag="qout")
                        nc.sync.dma_start(qout_t[:], qout_ap[:, :, n0:n0 + nw])

                    accs = [
                        acc_ps.tile([P, nw], f32, tag=f"acc{m}", name=f"acc{m}")
                        for m in range(4)
                    ]

                    # Software pipeline: mm1 pair + exp for pair pc, then the
                    # DoubleRow mm2 burst for pair pc-1 (so exp hides under
                    # PE's mm2 work).
                    for pc in range(NPAIR + 1):
                        if pc < NPAIR:
                            c0 = 2 * pc
                            s_t = s_ps.tile([P, 2, NW], f32, tag="s",
                                            padded_shape=[P, 2, 512])
                            nc.tensor.matmul(
                                s_t[:, 0, :nw], keys_sb[:, c0, :], qin_t[:],
                                start=True, stop=True,
                            )
                            if c0 + 1 < NB:
                                bw1 = P if c0 + 1 < NB - 1 else LAST_BW
                                nc.tensor.matmul(
                                    s_t[:bw1, 1, :nw], keys_sb[:, c0 + 1, :bw1],
                                    qin_t[:], start=True, stop=True,
                                )
                                nc.scalar.activation(
                                    e8_sb[:, c0:c0 + 2, :nw], s_t[:, :, :nw],
                                    Exp, scale=SCALE, bias=shift_sb[:, :])
                            else:
                                # last pair: chunk 56 is 32 banks; slot 57 and
                                # banks 32: of slot 56 are zero (one-time DMA)
                                nc.scalar.activation(
                                    e8_sb[:LAST_BW, c0:c0 + 1, :nw],
                                    s_t[:LAST_BW, 0:1, :nw],
                                    Exp, scale=SCALE, bias=shift_sb[:LAST_BW, :])
                        if pc > 0:
                            pq = pc - 1
                            pp = 2 * pq
                            for m in range(4):
                                nc.tensor.matmul(
                                    accs[m][:, :nw],
                                    (vals_f8[:, pq, m, :] if SW_ILV else
                                     vals_f8[:, pq, m, :].rearrange(
                                         "k (i q) -> k i q", i=2)),
                                    e8_sb[:, pp:pp + 2, :nw],
                                    start=(pc == 1), stop=(pc == NPAIR),
                                    perf_mode=DR,
                                )

                    # mask/denom accumulation sweep (PSUM slot from s-pool)
                    acc5_t = s_ps.tile([P, 2, NW], f32, tag="s", name="acc5",
                                       padded_shape=[P, 2, 512])
                    for pq in range(NPAIR):
                        pp = 2 * pq
                        nc.tensor.matmul(
                            acc5_t[0:16, 0, :nw] if SW_ILV else acc5_t[0:2, 0, :nw],
                            (mo_f8[:, pq, 0:32] if SW_ILV else
                             mo_f8[:, pq, :].rearrange(
                                 "k (i j) -> k i j", i=2)[:, :, 0:2]),
                            e8_sb[:, pp:pp + 2, :nw],
                            start=(pq == 0), stop=(pq == NPAIR - 1),
                            perf_mode=DR,
                        )

                    # Normalization: md2 rows = [maskraw, denom] (bf16)
                    md2 = row_p.tile([2, nw], bf16, tag="md2")
                    nc.scalar.copy(md2[:], acc5_t[0:2, 0, :nw])

                    bc_t = s_ps.tile([P, 2, NW], f32, tag="s", name="bc",
                                     padded_shape=[P, 2, 512])
                    nc.tensor.matmul(bc_t[:, 0, :nw], sel2[:], md2[:],
                                     start=True, stop=True)
                    rb_sb = bcsb_p.tile([P, nw], f32, tag="rb")
                    nc.vector.reciprocal(rb_sb[:], bc_t[:, 0, :nw])

                    nc.tensor.matmul(bc_t[:, 1, :nw], ones_col[:], md2[0:1, :],
                                     start=True, stop=True)
                    mn_sb = bcsb_p.tile([P, nw], f32, tag="mn")
                    nc.vector.tensor_mul(mn_sb[:], bc_t[:, 1, :nw], rb_sb[:])

                    for m in range(4):
                        o_t = out_p.tile([P, nw], bf16, tag="out")
                        nc.vector.tensor_mul(o_t[:], accs[m][:, :nw], rb_sb[:])
                        nc.sync.dma_start(out_ap[:, m, n0:n0 + nw], o_t[:])
                    for m in range(4):
                        o_t = out_p.tile([P, nw], bf16, tag="out")
                        nc.vector.tensor_mul(o_t[:], qout_t[:, m, :], mn_sb[:])
                        nc.sync.dma_start(out_ap[:, 4 + m, n0:n0 + nw], o_t[:])

            if bench:
                dsb = persist.tile([1, P], bf16)
                nc.vector.tensor_copy(dsb[:], ones_col[:])
                nc.sync.dma_start(dout_d.ap()[:, :], dsb[:])

    nc.compile()
    return nc


def _get_nc():
    if "nc" not in _CACHE:
        _CACHE["nc"] = _build()
    return _CACHE["nc"]


def _get_runner():
    """Build the multi-core PJRT runner once (mirrors bass2jax.run_bass_via_pjrt)."""
    if "runner" in _CACHE:
        return _CACHE["runner"]
    import jax
    from jax.sharding import Mesh, PartitionSpec
    from jax.experimental.shard_map import shard_map
    import concourse.mybir as mybir
    from concourse import bass2jax
    from concourse.bass2jax import _bass_exec_p, install_neuronx_cc_hook

    nc = _get_nc()
    install_neuronx_cc_hook()
    partition_name = nc.partition_id_tensor.name if nc.partition_id_tensor else None
    in_names, out_names, out_avals = [], [], []
    for alloc in nc.m.functions[0].allocations:
        if not isinstance(alloc, mybir.MemoryLocationSet):
            continue
        name = alloc.memorylocations[0].name
        if alloc.kind == "ExternalInput":
            if name != partition_name:
                in_names.append(name)
        elif alloc.kind == "ExternalOutput":
            out_names.append(name)
            out_avals.append(jax.core.ShapedArray(
                tuple(alloc.tensor_shape), mybir.dt.np(alloc.dtype)))
    n_params = len(in_names)
    zero_outs = [np.zeros(a.shape, a.dtype) for a in out_avals]
    all_in_names = list(in_names) + list(out_names)
    if partition_name is not None:
        all_in_names.append(partition_name)

    def _body(*args):
        operands = list(args)
        if partition_name is not None:
            operands.append(bass2jax.partition_id_tensor())
        outs = _bass_exec_p.bind(
            *operands,
            out_avals=tuple(out_avals),
            in_names=tuple(all_in_names),
            out_names=tuple(out_names),
            lowering_input_output_aliases=(),
            sim_require_finite=True,
            sim_require_nnan=True,
            nc=nc,
        )
        return tuple(outs)

    try:
        devices = jax.devices("axon")
    except Exception:
        devices = [d for d in jax.devices() if d.platform != "cpu"] or jax.devices()
    devices = devices[:N_CORES]
    assert len(devices) >= N_CORES, f"need {N_CORES} cores, got {len(devices)}"
    mesh = Mesh(np.asarray(devices), ("core",))
    n_io = n_params + len(out_names)
    fn = jax.jit(
        shard_map(_body, mesh=mesh,
                  in_specs=(PartitionSpec("core"),) * n_io,
                  out_specs=(PartitionSpec("core"),) * len(out_names),
                  check_rep=False),
        keep_unused=True)

    def run(in_maps):
        concat_in = [
            np.concatenate([np.asarray(m[name]) for m in in_maps], axis=0)
            for name in in_names
        ]
        concat_zero = [
            np.zeros((N_CORES * z.shape[0], *z.shape[1:]), z.dtype)
            for z in zero_outs
        ]
        out_arrs = fn(*concat_in, *concat_zero)
        return [
            {name: np.asarray(out_arrs[i]).reshape(N_CORES, *out_avals[i].shape)[c]
             for i, name in enumerate(out_names)}
            for c in range(N_CORES)
        ]

    _CACHE["runner"] = run
    return run


def kernel(keys, values, masks, q_in, q_out):

    keys = np.ascontiguousarray(np.asarray(keys, dtype=np.float32))
    values = np.asarray(values, dtype=np.float32)
    masks = np.asarray(masks, dtype=np.float32)
    q_in = np.ascontiguousarray(np.asarray(q_in, dtype=np.float32))
    q_out = np.asarray(q_out, dtype=np.float32)

    f8 = ml_dtypes.float8_e4m3
    bf = ml_dtypes.bfloat16

    # Host-side layout prep (per object, shared by 2 cores)
    keys_pad = np.zeros((OBJ_N, D_KEY, NB * P), dtype=np.float32)
    keys_pad[:, :, :BANK_N] = keys
    vpad = np.zeros((OBJ_N, D_VAL, B_PAD), dtype=f8)
    vpad[:, :, :BANK_N] = values.astype(f8)
    a = vpad.reshape(OBJ_N, 4, P, NPAIR, 2, P)   # [o, m, q, pc, i, p]
    if SW_ILV:
        # S[p, pc, m, 2j+i] = W_i[p, 127-j]: interleaved, columns reversed
        vals8 = a[:, :, ::-1, :, :, :].transpose(0, 5, 3, 1, 2, 4)
    else:
        # vals8[o, p, pc, m, i, q] = values[o, m*128+q, (2*pc+i)*128 + p]
        vals8 = a.transpose(0, 5, 3, 1, 4, 2)
    vals8 = np.ascontiguousarray(vals8).reshape(OBJ_N, P, NPAIR * 4 * 2 * P)
    vals8 = vals8.view(np.uint8)
    mpad = np.zeros((OBJ_N, 2, B_PAD), dtype=f8)
    mpad[:, 0, :BANK_N] = masks[:, 0].astype(f8)
    mpad[:, 1, :BANK_N] = 1.0
    mr = mpad.reshape(OBJ_N, 2, NPAIR, 2, P)     # [o, row(0=mask,1=ones), pc, i, p]
    mo8 = np.zeros((OBJ_N, P, NPAIR, 32), dtype=f8)
    if SW_ILV:
        # S[0]=ones_t0 S[1]=ones_t1 S[2]=mask_t0 S[3]=mask_t1 (cols reversed)
        mo8[:, :, :, 0] = mr[:, 1, :, 0, :].transpose(0, 2, 1)
        mo8[:, :, :, 1] = mr[:, 1, :, 1, :].transpose(0, 2, 1)
        mo8[:, :, :, 2] = mr[:, 0, :, 0, :].transpose(0, 2, 1)
        mo8[:, :, :, 3] = mr[:, 0, :, 1, :].transpose(0, 2, 1)
    else:
        # [pc, i, j] blocks at stride 16: j=0 mask, j=1 ones
        mo8.reshape(OBJ_N, P, NPAIR, 2, 16)[:, :, :, :, 0:2] = (
            mr.transpose(0, 4, 2, 3, 1))
    mo8 = np.ascontiguousarray(mo8).reshape(OBJ_N, P, NPAIR * 32)
    mo8 = mo8.view(np.uint8)
    zeros8 = np.zeros((P, 512), dtype=np.uint8)

    consts = np.zeros((2, P), dtype=bf)
    consts[1, :] = 1.0
    q_out_bf = q_out.astype(bf)

    in_maps = []
    for core in range(N_CORES):
        o, half = divmod(core, 2)
        nsl = slice(half * N_HALF, (half + 1) * N_HALF)
        in_maps.append({
            "consts": consts,
            "keys": keys_pad[o],
            "vals": vals8[o],
            "mo": mo8[o],
            "zeros": zeros8,
            "qin": np.ascontiguousarray(q_in[0, :, nsl]),
            "qout": np.ascontiguousarray(q_out_bf[0, :, nsl]),
        })

    run = _get_runner()
    results = run(in_maps)

    out = np.empty((1, OBJ_N, 2 * D_VAL, N_Q), dtype=np.float32)
    for core in range(N_CORES):
        o, half = divmod(core, 2)
        nsl = slice(half * N_HALF, (half + 1) * N_HALF)
        out[0, o, :, nsl] = results[core]["out"].astype(np.float32)
    return out
